# revision 6
# baseline (speedup 1.0000x reference)
"""Trainium2 Bass kernel for nn_AttnNeck (B=4, C=256, H=W=64).

out = gamma * (v @ softmax_n(x1^T x1)) + ref, with x1 = relu(conv3x3(ref, w1)),
v = relu(conv3x3(ref, w2)). The dead conv on `inputs` does not affect the
output and is skipped.

Sharding: 8 cores = 4 samples x 2 half-image shards. Odd cores receive the
sample rotated 180 degrees (and 180-rotated conv weights), which maps their
half (pixel rows 32..63) onto "rows 0..31 in rotated space" so every core
runs the identical static SPMD program. conv3x3/SAME commutes with rot180
on a square image, so results are exact.

Numerics: matmuls in float32r (~12-bit mantissa, measured max rel err
1.6e-4), softmax shifted by the Gram diagonal (== per-column max on these
inputs; exp args stay <= ~0 so no overflow regardless), E/v in bf16 for the
final contraction with the denominator computed from the same rounded E.
"""
import sys
sys.path.insert(0, '/opt/trn_rl_repo')

import numpy as np

B, C, H, W = 4, 256, 64, 64
HW = H * W          # 4096
MHALF = HW // 2     # 2048 columns per core
NCORES = 8
NBLK = MHALF // 512  # 4 m-blocks per core

_CACHE = {}


def _build(gamma: float):
    import concourse.bacc as bacc
    import concourse.mybir as mybir
    import concourse.tile as tile
    from concourse.masks import make_identity

    f32, f32r, bf16 = mybir.dt.float32, mybir.dt.float32r, mybir.dt.bfloat16
    AF = mybir.ActivationFunctionType
    ALU = mybir.AluOpType

    nc = bacc.Bacc("TRN2", target_bir_lowering=False, debug=False,
                   num_devices=NCORES)
    refp = nc.dram_tensor("refp", [C, H + 2, W + 2], f32, kind="ExternalInput")
    w1t = nc.dram_tensor("w1t", [2, 128, 9, C], f32, kind="ExternalInput")
    w2t = nc.dram_tensor("w2t", [2, 128, 9, C], f32, kind="ExternalInput")
    out = nc.dram_tensor("out", [C, MHALF], f32, kind="ExternalOutput")

    PW = W + 2  # 66
    NPAD = (H + 2) * PW  # 4356

    with tile.TileContext(nc) as tc:
        with tc.tile_pool(name="persist", bufs=1) as pers, \
             tc.tile_pool(name="work", bufs=3) as work:
            x1 = pers.tile([128, 2, HW], f32r)
            vT = pers.tile([128, 32, C], bf16)
            ident = pers.tile([128, 128], bf16)
            make_identity(nc, ident)
            ones_f = pers.tile([128, 1], f32)
            nc.vector.memset(ones_f, 1.0)
            ones_col = pers.tile([128, 1], f32r)
            nc.vector.tensor_copy(out=ones_col, in_=ones_f)
            ones_col_bf = pers.tile([128, 1], bf16)
            nc.vector.tensor_copy(out=ones_col_bf, in_=ones_f)
            ones_rf = pers.tile([1, 128], f32)
            nc.vector.memset(ones_rf, 1.0)
            ones_row = pers.tile([1, 128], f32r)
            nc.vector.tensor_copy(out=ones_row, in_=ones_rf)

            # ---------------- phase 1: convs ----------------
            with tc.tile_pool(name="convdat", bufs=1) as cd, \
                 tc.tile_pool(name="stage", bufs=2) as stage, \
                 tc.tile_pool(name="convps", bufs=4, space="PSUM") as cps, \
                 tc.tile_pool(name="trps", bufs=3, space="PSUM") as tps:
                ref_sb = cd.tile([128, 2, NPAD], f32r)
                w1r = cd.tile([128, 2, 9, C], f32r)
                w2r = cd.tile([128, 2, 9, C], f32r)
                v = cd.tile([128, 2, HW], bf16)

                for cc in range(2):
                    st = stage.tile([128, NPAD], f32, tag="st")
                    nc.sync.dma_start(
                        out=st,
                        in_=refp[cc * 128:(cc + 1) * 128, :, :].rearrange(
                            "p a b -> p (a b)"))
                    nc.vector.tensor_copy(out=ref_sb[:, cc, :], in_=st)
                for wt, wr in ((w1t, w1r), (w2t, w2r)):
                    for cc in range(2):
                        st = stage.tile([128, 9 * C], f32, tag="st")
                        nc.sync.dma_start(
                            out=st,
                            in_=wt[cc, :, :, :].rearrange("p a b -> p (a b)"))
                        nc.vector.tensor_copy(
                            out=wr[:, cc, :, :].rearrange("p a b -> p (a b)"),
                            in_=st)

                ref_rows = [ref_sb[:, ic, :].rearrange("p (r c) -> p r c", c=PW)
                            for ic in range(2)]

                def conv(wr, out_cb, out_dtype_relu):
                    # out_cb(cc, blk, psum) consumes the relu'd psum
                    for cc in range(2):
                        for blk in range(8):
                            ps = cps.tile([128, 512], mybir.dt.float32,
                                          tag="cv")
                            k = 0
                            for t in range(9):
                                dy, dx = t // 3 - 1, t % 3 - 1
                                r0 = 8 * blk + dy + 1
                                x0 = dx + 1
                                for ic in range(2):
                                    nc.tensor.matmul(
                                        ps,
                                        wr[:, ic, t, cc * 128:(cc + 1) * 128],
                                        ref_rows[ic][:, r0:r0 + 8, x0:x0 + W],
                                        start=(k == 0), stop=(k == 17))
                                    k += 1
                            out_cb(cc, blk, ps)

                def x1_out(cc, blk, ps):
                    nc.scalar.activation(
                        out=x1[:, cc, blk * 512:(blk + 1) * 512], in_=ps,
                        func=AF.Relu)

                def v_out(cc, blk, ps):
                    nc.scalar.activation(
                        out=v[:, cc, blk * 512:(blk + 1) * 512], in_=ps,
                        func=AF.Relu)

                conv(w1r, x1_out, None)
                conv(w2r, v_out, None)

                # vT via PE transposes (bf16)
                for cc in range(2):
                    for j in range(32):
                        pt = tps.tile([128, 128], bf16, tag="tr")
                        nc.tensor.transpose(
                            pt, v[:, cc, j * 128:(j + 1) * 128], ident)
                        nc.vector.tensor_copy(
                            out=vT[:, j, cc * 128:(cc + 1) * 128], in_=pt)

            # ---------------- phase 2: diag + attention ----------------
            with tc.tile_pool(name="attn", bufs=1) as at, \
                 tc.tile_pool(name="sblk", bufs=3) as sblk, \
                 tc.tile_pool(name="oblk", bufs=4) as oblk, \
                 tc.tile_pool(name="sps", bufs=3, space="PSUM") as sps, \
                 tc.tile_pool(name="aps", bufs=1, space="PSUM") as aps, \
                 tc.tile_pool(name="dps", bufs=1, space="PSUM") as dps, \
                 tc.tile_pool(name="bps", bufs=1, space="PSUM") as bps:
                E = at.tile([128, 32, 512], bf16)
                bcast_diag = at.tile([128, NBLK, 512], f32)
                sq = at.tile([128, 2, MHALF], f32r)

                for ic in range(2):
                    nc.vector.tensor_mul(sq[:, ic, :], x1[:, ic, :MHALF],
                                         x1[:, ic, :MHALF])

                for j in range(NBLK):
                    pd = dps.tile([1, 512], mybir.dt.float32, tag="diag")
                    nc.tensor.matmul(pd, ones_col,
                                     sq[:, 0, j * 512:(j + 1) * 512],
                                     start=True, stop=False)
                    nc.tensor.matmul(pd, ones_col,
                                     sq[:, 1, j * 512:(j + 1) * 512],
                                     start=False, stop=True)
                    dr = sblk.tile([1, 512], f32r, tag="dr")
                    nc.vector.tensor_copy(out=dr, in_=pd)
                    pb = bps.tile([128, 512], mybir.dt.float32, tag="bc")
                    nc.tensor.matmul(pb, ones_row, dr, start=True, stop=True)
                    nc.vector.tensor_copy(out=bcast_diag[:, j, :], in_=pb)

                f32_ = mybir.dt.float32
                for j in range(NBLK):
                    mlo = j * 512
                    # scores + exp into E
                    for nt in range(32):
                        ps = sps.tile([128, 512], f32_, tag="sc")
                        nc.tensor.matmul(
                            ps, x1[:, 0, nt * 128:(nt + 1) * 128],
                            x1[:, 0, mlo:mlo + 512], start=True, stop=False)
                        nc.tensor.matmul(
                            ps, x1[:, 1, nt * 128:(nt + 1) * 128],
                            x1[:, 1, mlo:mlo + 512], start=False, stop=True)
                        sh = sblk.tile([128, 512], f32_, tag="sh")
                        nc.vector.scalar_tensor_tensor(
                            out=sh, in0=ps, scalar=1.0,
                            in1=bcast_diag[:, j, :],
                            op0=ALU.mult, op1=ALU.subtract)
                        nc.scalar.activation(out=E[:, nt, :], in_=sh,
                                             func=AF.Exp)
                    # A = vT^T @ E accumulated over n; D = ones^T @ E
                    pa0 = aps.tile([128, 512], f32_, tag="a0")
                    pa1 = aps.tile([128, 512], f32_, tag="a1")
                    pdn = dps.tile([1, 512], f32_, tag="diag")
                    for nt in range(32):
                        st_, sp_ = (nt == 0), (nt == 31)
                        nc.tensor.matmul(pa0, vT[:, nt, 0:128], E[:, nt, :],
                                         start=st_, stop=sp_)
                        nc.tensor.matmul(pa1, vT[:, nt, 128:256], E[:, nt, :],
                                         start=st_, stop=sp_)
                        nc.tensor.matmul(pdn, ones_col_bf, E[:, nt, :],
                                         start=st_, stop=sp_)
                    rD = sblk.tile([1, 512], f32_, tag="rD")
                    nc.vector.reciprocal(out=rD, in_=pdn)
                    rDr = sblk.tile([1, 512], f32r, tag="rDr")
                    nc.vector.tensor_copy(out=rDr, in_=rD)
                    pb = bps.tile([128, 512], f32_, tag="bc")
                    nc.tensor.matmul(pb, ones_row, rDr, start=True, stop=True)
                    pbs = oblk.tile([128, 512], f32_, tag="pbs")
                    nc.vector.tensor_copy(out=pbs, in_=pb)
                    for cc, pa in ((0, pa0), (1, pa1)):
                        reff = oblk.tile([128, 8, W], f32, tag="reff")
                        nc.sync.dma_start(
                            out=reff,
                            in_=refp[cc * 128:(cc + 1) * 128,
                                     1 + 8 * j:9 + 8 * j, 1:1 + W])
                        tmp = oblk.tile([128, 512], f32_, tag="tmp")
                        nc.vector.tensor_mul(tmp, pa, pbs)
                        ot = oblk.tile([128, 512], f32_, tag="ot")
                        nc.vector.scalar_tensor_tensor(
                            out=ot, in0=tmp, scalar=float(gamma),
                            in1=reff.rearrange("p a b -> p (a b)"),
                            op0=ALU.mult, op1=ALU.add)
                        nc.sync.dma_start(
                            out=out[cc * 128:(cc + 1) * 128,
                                    mlo:mlo + 512], in_=ot)

    nc.compile()
    return nc


def _make_runner(nc):
    import jax
    from jax.sharding import Mesh, PartitionSpec
    from jax.experimental.shard_map import shard_map
    import concourse.mybir as mybir
    from concourse.bass2jax import (_bass_exec_p, install_neuronx_cc_hook,
                                    partition_id_tensor)

    install_neuronx_cc_hook()
    partition_name = (nc.partition_id_tensor.name
                      if nc.partition_id_tensor else None)
    in_names, out_names, out_avals, zero_outs = [], [], [], []
    for alloc in nc.m.functions[0].allocations:
        if not isinstance(alloc, mybir.MemoryLocationSet):
            continue
        name = alloc.memorylocations[0].name
        if alloc.kind == "ExternalInput":
            if name != partition_name:
                in_names.append(name)
        elif alloc.kind == "ExternalOutput":
            shape = tuple(alloc.tensor_shape)
            dtype = mybir.dt.np(alloc.dtype)
            out_avals.append(jax.core.ShapedArray(shape, dtype))
            out_names.append(name)
            zero_outs.append(np.zeros(shape, dtype))
    n_params = len(in_names)
    n_outs = len(out_avals)
    all_in_names = list(in_names) + list(out_names)
    if partition_name is not None:
        all_in_names.append(partition_name)

    def _body(*args):
        operands = list(args)
        if partition_name is not None:
            operands.append(partition_id_tensor())
        return tuple(_bass_exec_p.bind(
            *operands, out_avals=tuple(out_avals),
            in_names=tuple(all_in_names), out_names=tuple(out_names),
            lowering_input_output_aliases=(),
            sim_require_finite=True, sim_require_nnan=True, nc=nc))

    devices = jax.devices()[:NCORES]
    mesh = Mesh(np.asarray(devices), ("core",))
    jitted = jax.jit(
        shard_map(_body, mesh=mesh,
                  in_specs=(PartitionSpec("core"),) * (n_params + n_outs),
                  out_specs=(PartitionSpec("core"),) * n_outs,
                  check_rep=False),
        keep_unused=True)

    def run(in_maps):
        import jax as _jax
        per_core = [[np.asarray(m[n]) for n in in_names] for m in in_maps]
        concat_in = [
            np.ascontiguousarray(
                np.concatenate([per_core[c][i] for c in range(NCORES)],
                               axis=0))
            for i in range(n_params)
        ]
        concat_zeros = [
            np.zeros((NCORES * z.shape[0], *z.shape[1:]), z.dtype)
            for z in zero_outs
        ]
        outs = jitted(*concat_in, *concat_zeros)
        _jax.block_until_ready(outs)
        return [
            {n: np.asarray(outs[i]).reshape(NCORES, *out_avals[i].shape)[c]
             for i, n in enumerate(out_names)}
            for c in range(NCORES)
        ]

    return run


def _prep_weights(w):
    # w: [O=256, I=256, 3, 3] -> [2, 128, 9, 256]  ([cin_chunk, cin, tap, cout])
    wt = np.transpose(w, (1, 2, 3, 0)).reshape(C, 9, C)  # [cin, tap, cout]
    return np.ascontiguousarray(
        wt.reshape(2, 128, 9, C)).astype(np.float32)


def make_in_maps(inputs_np, ref_np, w1_np, w2_np):
    w1t = _prep_weights(w1_np)
    w2t = _prep_weights(w2_np)
    w1tr = _prep_weights(w1_np[:, :, ::-1, ::-1])
    w2tr = _prep_weights(w2_np[:, :, ::-1, ::-1])
    in_maps = []
    for core in range(NCORES):
        b, rot = core // 2, core % 2
        r = ref_np[b]
        if rot:
            r = r[:, ::-1, ::-1]
        rp = np.zeros((C, H + 2, W + 2), np.float32)
        rp[:, 1:H + 1, 1:W + 1] = r
        in_maps.append({
            "refp": np.ascontiguousarray(rp),
            "w1t": w1tr if rot else w1t,
            "w2t": w2tr if rot else w2t,
        })
    return in_maps


def assemble(results, ref_np, gamma):
    full = np.empty((B, C, HW), np.float32)
    for core in range(NCORES):
        b, rot = core // 2, core % 2
        o = results[core]["out"]  # [C, MHALF]
        if rot:
            full[b][:, MHALF:] = o[:, ::-1]
        else:
            full[b][:, :MHALF] = o
    return full.reshape(B, C, H, W)


def kernel(inputs, ref, w1, w2, gamma):
    inputs = np.asarray(inputs, np.float32)
    ref = np.asarray(ref, np.float32)
    w1 = np.asarray(w1, np.float32)
    w2 = np.asarray(w2, np.float32)
    g = float(np.asarray(gamma))
    key = ("k", g)
    if key not in _CACHE:
        nc = _build(g)
        _CACHE[("nc", g)] = nc
        _CACHE[key] = _make_runner(nc)
    run = _CACHE[key]
    in_maps = make_in_maps(inputs, ref, w1, w2)
    results = run(in_maps)
    return assemble(results, ref, g)
